# revision 19
# baseline (speedup 1.0000x reference)
"""Trainium2 Bass kernel for nn_MeshMultiHeadAttention_6408091206165.

Sharding: 16 (batch, head) units over 8 cores -> core c handles batch c//4 and
heads {2*(c%4), 2*(c%4)+1}. Per-head attention terms are independent; the
merge+output-linear is computed per-head on device (linear in heads) and the
per-core partial outputs are summed on the host.

Structural optimizations:
- d_0 (2 nonzeros/row) and d_1 (<=3 nonzeros/row) are boundary operators; all
  three boundary applications are row-gathers. The input-side lifts
  (d_0 @ x_v-space, d_1 @ x_e-space) are computed on the host during input
  sharding (exact: same values as the dense matmul, which only adds zeros).
  The output-side application (d_0 @ P_v) is applied during the host unshard
  (it is linear, so it commutes with the head-sum).
- LayerNorm mean subtraction is folded into the projection weights on the host
  (centered weights: W~ = (I - 11^T/32) @ W), so the device only needs the
  variance (computed via a ones-matmul on the squared projections).
- Softmax skips the max-subtraction (logits are O(30), exp stays in fp32
  range) and the denominator is accumulated by an extra ones-column in the
  attention-apply matmul; normalization is applied to the tiny per-head
  attention outputs instead of the big S matrices. The junk denominator rows
  are killed by zero rows in the output-projection weights.

All tensors fp32. Layout: Q/K tensors are feature-major [64, n] with head h at
rows 32h:32h+32 (matches lhsT/rhs base-partition pairing); V/lift tensors are
token-major [128, tiles, 66] with per-head column windows [V(32)|ones].
"""
import math
from functools import lru_cache

import numpy as np

import concourse.bass as bass  # noqa: F401
import concourse.mybir as mybir
import concourse.tile as tile
from concourse import bacc
from concourse.bass_utils import run_bass_kernel_spmd

H, DV, DK = 8, 256, 32
BS, N, M, K = 2, 1024, 3072, 2048
EPS = 1e-5
ISQ = 1.0 / math.sqrt(DK)
FP = mybir.dt.float32
Act = mybir.ActivationFunctionType

FR = mybir.dt.float32r  # full-rate PE matmul dtype (tf32-like rounding)

INPUT_SPECS = [
    ("xvT", [DV, N]), ("xeT", [DV, M]), ("xfT", [DV, K]),
    ("gvT", [DV, M]), ("geT", [DV, K]),
    ("w_vQ", [DV, 64]), ("w_vK", [DV, 64]), ("w_eQ", [DV, 64]),
    ("w_eK", [DV, 64]), ("w_eV", [DV, 64]), ("w_fK", [DV, 64]),
    ("w_lv", [DV, 64]), ("w_ov", [128, DV]), ("w_oe", [128, DV]),
    ("g_vQ", [64, 1]), ("b_vQ", [64, 1]), ("g_vK", [64, 1]), ("b_vK", [64, 1]),
    ("g_eQ", [64, 1]), ("b_eQ", [64, 1]), ("g_eK", [64, 1]), ("b_eK", [64, 1]),
    ("g_fK", [64, 1]), ("b_fK", [64, 1]),
    ("ones_sq", [64, 2]), ("bc2", [2, 64]), ("bc128", [2, 128]),
]

NT = N // 128   # 8  vertex tiles
MT = M // 128   # 24 edge tiles
KT = K // 128   # 16 face tiles


def _build_bass():
    nc = bacc.Bacc("TRN2", target_bir_lowering=False, debug=False)

    def din(name, shape, dt=FP):
        return nc.dram_tensor(name, shape, dt, kind="ExternalInput")

    dram = {nm: din(nm, shape) for nm, shape in INPUT_SPECS}
    dram["outv_d"] = nc.dram_tensor("outv", [N, DV], FP, kind="ExternalOutput")
    dram["pv_d"] = nc.dram_tensor("pv", [N, DV], FP, kind="ExternalOutput")
    dram["oute_d"] = nc.dram_tensor("oute", [M, DV], FP, kind="ExternalOutput")

    with tile.TileContext(nc) as tc:
        _emit(nc, tc, dram)
    nc.compile()
    return nc


def _emit(nc, tc, t):
    import contextlib
    ctx = contextlib.ExitStack()
    with ctx:
        ctx.enter_context(nc.allow_low_precision(
            reason="float32r (tf32-like) matmul inputs are intentional; psum accumulation stays fp32"))
        wp = ctx.enter_context(tc.tile_pool(name="wp", bufs=1))
        accp = ctx.enter_context(tc.tile_pool(name="accp", bufs=1))
        ps = ctx.enter_context(tc.tile_pool(name="ps", bufs=2, space="PSUM"))
        projctx = contextlib.ExitStack()
        proj = projctx.enter_context(tc.tile_pool(name="proj", bufs=1))

        # ---- load weights/constants ----
        def wload(name, shape, rearr=None, dt=None):
            tl = wp.tile(shape, dt or FP, tag=name)
            src = t[name]
            eng = nc.gpsimd if (dt or FP) == FR else nc.sync
            eng.dma_start(out=tl, in_=src.rearrange(rearr, p=128) if rearr else src[:, :])
            return tl

        w_vQ = wload("w_vQ", [128, 2, 64], "(t p) n -> p t n", dt=FR)
        w_vK = wload("w_vK", [128, 2, 64], "(t p) n -> p t n", dt=FR)
        w_eQ = wload("w_eQ", [128, 2, 64], "(t p) n -> p t n", dt=FR)
        w_eK = wload("w_eK", [128, 2, 64], "(t p) n -> p t n", dt=FR)
        w_eV = wload("w_eV", [128, 2, 64], "(t p) n -> p t n", dt=FR)
        w_fK = wload("w_fK", [128, 2, 64], "(t p) n -> p t n", dt=FR)
        w_lv = wload("w_lv", [128, 2, 64], "(t p) n -> p t n", dt=FR)
        w_ov = wload("w_ov", [128, 256], dt=FR)
        w_oe = wload("w_oe", [128, 256], dt=FR)
        lnp = {nm: wload(nm, [64, 1])
               for nm in ("g_vQ", "b_vQ", "g_vK", "b_vK", "g_eQ", "b_eQ",
                          "g_eK", "b_eK", "g_fK", "b_fK")}
        ones_sq = wload("ones_sq", [64, 2], dt=FR)
        bc2 = wload("bc2", [2, 64], dt=FR)
        bc128 = wload("bc128", [2, 128], dt=FR)
        epsv = wp.tile([2, 1], FP, tag="epsv")
        nc.vector.memset(epsv, EPS)

        # ---- projection outputs (live through attention phase) ----
        vQn = proj.tile([64, N], FR, tag="vQn")
        vKn = proj.tile([64, N], FR, tag="vKn")
        eQn = proj.tile([64, M], FR, tag="eQn")
        eKn = proj.tile([64, M], FR, tag="eKn")
        fKn = proj.tile([64, K], FR, tag="fKn")
        eVo = proj.tile([128, MT, 128], FR, tag="eVo")  # cols [Vh0|1|0..|Vh1|1|0..]
        Lvo = proj.tile([128, MT, 128], FR, tag="Lvo")
        Leo = proj.tile([128, KT, 128], FR, tag="Leo")

        # =========== Phase A: projections + LN (inputs streamed in 1K chunks) ===========
        U32 = mybir.dt.uint32
        ONE_BITS = 0x3F800000
        for vt in (eVo, Lvo, Leo):
            nc.vector.memset(vt.bitcast(U32), 0)
            nc.vector.memset(vt[:, :, 32:33].bitcast(U32), ONE_BITS)
            nc.vector.memset(vt[:, :, 96:97].bitcast(U32), ONE_BITS)
        with tc.tile_pool(name="inp", bufs=2) as inp, \
             tc.tile_pool(name="tmp", bufs=2) as tmp:

            def ln_chunk(xc, w, out_tile, c, g_ap, b_ap):
                # centered projection -> y~ ; var = ones_sq @ y~^2 ;
                # inv = 1/sqrt(var+eps) ; PE-broadcast ; y~ * inv * g + b
                cs = slice(c * 1024, (c + 1) * 1024)
                pb = ps.tile([128, 1024], FP, tag="pb")
                for u in range(2):
                    for k in range(2):
                        nc.tensor.matmul(
                            pb[:64, u * 512:(u + 1) * 512], w[:, k, :],
                            xc[:, k, u * 512:(u + 1) * 512],
                            start=(k == 0), stop=(k == 1))
                y = tmp.tile([64, 1024], FP, tag="y")
                y2 = tmp.tile([64, 1024], FR, tag="y2")
                nc.vector.tensor_copy(y, pb[:64])
                nc.scalar.square(y2, pb[:64])
                pstat = ps.tile([2, 1024], FP, tag="paV", bufs=1)
                for u in range(2):
                    nc.tensor.matmul(pstat[:, u * 512:(u + 1) * 512], ones_sq,
                                     y2[:, u * 512:(u + 1) * 512],
                                     start=True, stop=True)
                sd = tmp.tile([2, 1024], FP, tag="sd", bufs=1)
                nc.scalar.activation(sd, pstat, Act.Sqrt, bias=epsv)
                inv = tmp.tile([2, 1024], FR, tag="inv", bufs=1)
                nc.vector.reciprocal(inv, sd)
                pbc = ps.tile([128, 1024], FP, tag="pb")
                for u in range(2):
                    nc.tensor.matmul(pbc[:64, u * 512:(u + 1) * 512], bc2,
                                     inv[:, u * 512:(u + 1) * 512],
                                     start=True, stop=True)
                sc = tmp.tile([64, 1024], FP, tag="sc")
                nc.vector.tensor_mul(sc, y, pbc[:64])
                nc.scalar.activation(out_tile[:, cs], sc, Act.Identity,
                                     bias=b_ap, scale=g_ap)

            def nat_chunk(xc, w, out_tile, c):
                for i in range(8):
                    mt = 8 * c + i
                    pv = ps.tile([128, 64], FP, tag="pb")
                    for k in range(2):
                        nc.tensor.matmul(pv, xc[:, k, i * 128:(i + 1) * 128],
                                         w[:, k, :], start=(k == 0), stop=(k == 1))
                    nc.vector.tensor_copy(
                        out_tile[:, mt, :].rearrange("p (s c) -> p s c", s=2)[:, :, 0:32],
                        pv.rearrange("p (s c) -> p s c", s=2))

            def load_chunk(nm, c):
                xc = inp.tile([128, 2, 1024], FR, tag="xc")
                nc.gpsimd.dma_start(
                    out=xc,
                    in_=t[nm].rearrange("(t p) n -> p t n", p=128)[:, :, c * 1024:(c + 1) * 1024])
                return xc

            for c in range(N // 1024):
                xc = load_chunk("xvT", c)
                ln_chunk(xc, w_vQ, vQn, c, lnp["g_vQ"], lnp["b_vQ"])
                ln_chunk(xc, w_vK, vKn, c, lnp["g_vK"], lnp["b_vK"])
            for c in range(M // 1024):
                xc = load_chunk("xeT", c)
                ln_chunk(xc, w_eQ, eQn, c, lnp["g_eQ"], lnp["b_eQ"])
                ln_chunk(xc, w_eK, eKn, c, lnp["g_eK"], lnp["b_eK"])
                nat_chunk(xc, w_eV, eVo, c)
            for c in range(K // 1024):
                xc = load_chunk("xfT", c)
                ln_chunk(xc, w_fK, fKn, c, lnp["g_fK"], lnp["b_fK"])
            for c in range(M // 1024):
                xc = load_chunk("gvT", c)
                nat_chunk(xc, w_lv, Lvo, c)
            for c in range(K // 1024):
                xc = load_chunk("geT", c)
                nat_chunk(xc, w_eV, Leo, c)

        # =========== Phase B: attention ===========
        # f32r matmuls cannot write psum at a nonzero base partition, so each
        # head accumulates into a base-0 [64, n] psum and a partition-moving
        # DMA places it at rows 64h of the fp32 accumulator tiles.
        accV = accp.tile([128, N], FP, tag="accV")  # rows [xv_h0|d1_h0|0|xv_h1|d1_h1|0]
        accA = accp.tile([128, N], FP, tag="accA")  # same for Av / den3
        acc2 = accp.tile([128, M], FP, tag="acc2")  # xef / den2
        with tc.tile_pool(name="spool", bufs=4) as spool:
            for h in range(2):
                hs = slice(32 * h, 32 * h + 32)
                vs = slice(64 * h, 64 * h + 64)
                paV = ps.tile([64, N], FP, tag="paV", bufs=1)
                paA = ps.tile([64, N], FP, tag="paA", bufs=1)
                for mt in range(MT):
                    ekh = eKn[hs, mt * 128:(mt + 1) * 128]
                    pb1 = ps.tile([128, N], FP, tag="pb")
                    pb3 = ps.tile([128, N], FP, tag="pb")
                    for c in range(2):
                        nc.tensor.matmul(pb1[:, c * 512:(c + 1) * 512], ekh,
                                         vQn[hs, c * 512:(c + 1) * 512],
                                         start=True, stop=True)
                    for c in range(2):
                        nc.tensor.matmul(pb3[:, c * 512:(c + 1) * 512], ekh,
                                         vKn[hs, c * 512:(c + 1) * 512],
                                         start=True, stop=True)
                    s1 = spool.tile([128, N], FR, tag="s")
                    s3 = spool.tile([128, N], FR, tag="s")
                    nc.scalar.activation(s1, pb1, Act.Exp, scale=ISQ)
                    nc.scalar.activation(s3, pb3, Act.Exp, scale=ISQ)
                    for c in range(2):
                        nc.tensor.matmul(paV[:, c * 512:(c + 1) * 512],
                                         Lvo[:, mt, vs], s1[:, c * 512:(c + 1) * 512],
                                         start=(mt == 0), stop=(mt == MT - 1))
                    for c in range(2):
                        nc.tensor.matmul(paA[:, c * 512:(c + 1) * 512],
                                         eVo[:, mt, vs], s3[:, c * 512:(c + 1) * 512],
                                         start=(mt == 0), stop=(mt == MT - 1))
                if h == 0:
                    nc.vector.tensor_copy(accV[0:64, :], paV)
                    nc.vector.tensor_copy(accA[0:64, :], paA)
                else:
                    tV = spool.tile([64, N], FP, tag="tmv")
                    tA = spool.tile([64, N], FP, tag="tmv")
                    nc.vector.tensor_copy(tV, paV)
                    nc.vector.tensor_copy(tA, paA)
                    nc.sync.dma_start(out=accV[64:128, :], in_=tV)
                    nc.sync.dma_start(out=accA[64:128, :], in_=tA)

            for mc in range(3):
                for h in range(2):
                    hs = slice(32 * h, 32 * h + 32)
                    vs = slice(64 * h, 64 * h + 64)
                    pa2 = ps.tile([64, 1024], FP, tag="paV", bufs=1)
                    for kt in range(KT):
                        fkh = fKn[hs, kt * 128:(kt + 1) * 128]
                        pb2 = ps.tile([128, 1024], FP, tag="pb")
                        for c in range(2):
                            nc.tensor.matmul(
                                pb2[:, c * 512:(c + 1) * 512], fkh,
                                eQn[hs, mc * 1024 + c * 512: mc * 1024 + (c + 1) * 512],
                                start=True, stop=True)
                        s2 = spool.tile([128, 1024], FR, tag="s")
                        nc.scalar.activation(s2, pb2, Act.Exp, scale=ISQ)
                        for c in range(2):
                            nc.tensor.matmul(pa2[:, c * 512:(c + 1) * 512],
                                             Leo[:, kt, vs], s2[:, c * 512:(c + 1) * 512],
                                             start=(kt == 0), stop=(kt == KT - 1))
                    if h == 0:
                        nc.vector.tensor_copy(acc2[0:64, mc * 1024:(mc + 1) * 1024], pa2)
                    else:
                        t2 = spool.tile([64, 1024], FP, tag="tmv")
                        nc.vector.tensor_copy(t2, pa2)
                        nc.sync.dma_start(out=acc2[64:128, mc * 1024:(mc + 1) * 1024], in_=t2)

        projctx.close()

        # =========== Phase C: normalize + merge + output linears ===========
        with tc.tile_pool(name="outp", bufs=1) as outp:
            dbV = outp.tile([2, N], FR, tag="dbV")   # den1 h0, h1
            dbA = outp.tile([2, N], FR, tag="dbA")   # den3 h0, h1
            db2 = outp.tile([2, M], FR, tag="db2")   # den2 h0, h1
            for h in range(2):
                nc.gpsimd.dma_start(out=dbV[h:h + 1, :], in_=accV[64 * h + 32:64 * h + 33, :])
                nc.gpsimd.dma_start(out=dbA[h:h + 1, :], in_=accA[64 * h + 32:64 * h + 33, :])
                nc.gpsimd.dma_start(out=db2[h:h + 1, :], in_=acc2[64 * h + 32:64 * h + 33, :])

            def recip_bcast(db_rows, n_cols, tag):
                out = outp.tile([128, n_cols], FP, tag=tag)
                for c0 in range(0, n_cols, 1024):
                    pr = ps.tile([128, 1024], FP, tag="paV", bufs=1)
                    for u in range(2):
                        nc.tensor.matmul(
                            pr[:, u * 512:(u + 1) * 512], bc128,
                            db_rows[:, c0 + u * 512: c0 + (u + 1) * 512],
                            start=True, stop=True)
                    nc.vector.reciprocal(out[:, c0:c0 + 1024], pr)
                return out

            RV = recip_bcast(dbV, N, "RV")
            RA = recip_bcast(dbA, N, "RA")
            R2 = recip_bcast(db2, M, "R2")

            stv = outp.tile([128, N], FR, tag="stv")
            stp = outp.tile([128, N], FR, tag="stp")
            st2 = outp.tile([128, M], FR, tag="st2")
            nc.vector.tensor_mul(stv, accV, RV)
            nc.vector.tensor_mul(stp, accA, RA)
            nc.vector.tensor_mul(st2, acc2, R2)

            outv = outp.tile([128, NT, 256], FP, tag="outv")
            pvt = outp.tile([128, NT, 256], FP, tag="pvt")
            oute = outp.tile([128, MT, 256], FP, tag="oute")
            for nt in range(NT):
                po = ps.tile([128, 256], FP, tag="pb")
                nc.tensor.matmul(po, stv[:, nt * 128:(nt + 1) * 128], w_ov,
                                 start=True, stop=True)
                nc.scalar.copy(outv[:, nt, :], po)
                po2 = ps.tile([128, 256], FP, tag="pb")
                nc.tensor.matmul(po2, stp[:, nt * 128:(nt + 1) * 128], w_oe,
                                 start=True, stop=True)
                nc.scalar.copy(pvt[:, nt, :], po2)
            for mt in range(MT):
                po = ps.tile([128, 256], FP, tag="pb")
                nc.tensor.matmul(po, st2[:, mt * 128:(mt + 1) * 128], w_oe,
                                 start=True, stop=True)
                nc.scalar.copy(oute[:, mt, :], po)

            nc.sync.dma_start(out=t["outv_d"].rearrange("(t p) d -> p t d", p=128),
                              in_=outv)
            nc.sync.dma_start(out=t["pv_d"].rearrange("(t p) d -> p t d", p=128),
                              in_=pvt)
            nc.sync.dma_start(out=t["oute_d"].rearrange("(t p) d -> p t d", p=128),
                              in_=oute)


@lru_cache(maxsize=1)
def _get_nc():
    return _build_bass()


def _kernel_body(nc, handles):
    t = dict(zip([nm for nm, _ in INPUT_SPECS], handles))
    t["outv_d"] = nc.dram_tensor("outv", [N, DV], FP, kind="ExternalOutput")
    t["pv_d"] = nc.dram_tensor("pv", [N, DV], FP, kind="ExternalOutput")
    t["oute_d"] = nc.dram_tensor("oute", [M, DV], FP, kind="ExternalOutput")
    with tile.TileContext(nc) as tc:
        _emit(nc, tc, t)
    return (t["outv_d"], t["pv_d"], t["oute_d"])


@lru_cache(maxsize=1)
def _get_jax_fn():
    import jax
    from concourse.bass2jax import bass_jit
    body = bass_jit(_kernel_body)
    return jax.jit(body)


def _prep_host(inputs):
    """Build the 8 per-core input maps + host-side gather indices."""
    xs = {k: np.asarray(v) for k, v in inputs.items()}
    x_v, x_e, x_f = xs["x_v"], xs["x_e"], xs["x_f"]
    d_0, d_1 = xs["d_0"], xs["d_1"]

    gidx = []
    for b in range(BS):
        ip = np.argmax(d_0[b] == 1.0, axis=1)
        im = np.argmax(d_0[b] == -1.0, axis=1)
        gidx.append((ip, im))

    batch_shared = []
    for b in range(BS):
        ip, im = gidx[b]
        G_v = x_v[b][ip] - x_v[b][im]
        rows, cols = np.nonzero(d_1[b])
        vals = d_1[b][rows, cols]
        G_e = np.zeros((K, DV), np.float32)
        np.add.at(G_e, rows, (vals[:, None] * x_e[b][cols]).astype(np.float32))
        batch_shared.append(dict(
            xvT=np.ascontiguousarray(x_v[b].T),
            xeT=np.ascontiguousarray(x_e[b].T),
            xfT=np.ascontiguousarray(x_f[b].T),
            gvT=np.ascontiguousarray(G_v.T.astype(np.float32)),
            geT=np.ascontiguousarray(G_e.T),
        ))

    ones_sq = np.zeros((64, 2), np.float32)
    for j in range(2):
        ones_sq[32 * j:32 * j + 32, j] = 1.0 / 32.0
    bc2 = np.zeros((2, 64), np.float32)
    for j in range(2):
        bc2[j, 32 * j:32 * j + 32] = 1.0
    bc128 = np.zeros((2, 128), np.float32)
    bc128[0, 0:64] = 1.0
    bc128[1, 64:128] = 1.0

    in_maps = []
    for c in range(8):
        b, hp = divmod(c, 4)
        h0, h1 = 2 * hp, 2 * hp + 1

        def pair_T(w, center):
            blocks = []
            for h in (h0, h1):
                blk = np.asarray(w)[32 * h:32 * h + 32, :].astype(np.float64)
                if center:
                    blk = blk - blk.mean(axis=0, keepdims=True)
                blocks.append(blk.T)
            return np.concatenate(blocks, axis=1).astype(np.float32)

        def out_w(w):
            r = np.zeros((128, DV), np.float32)
            for i, h in enumerate((h0, h1)):
                r[64 * i:64 * i + 32] = np.asarray(w)[:, 32 * h:32 * h + 32].T
            return r

        def lnvec(nm):
            g = np.concatenate([np.asarray(xs["g_" + nm])[h] for h in (h0, h1)])
            bv = np.concatenate([np.asarray(xs["b_" + nm])[h] for h in (h0, h1)])
            return g.astype(np.float32)[:, None], bv.astype(np.float32)[:, None]

        m = dict(batch_shared[b])
        m["w_vQ"] = pair_T(xs["vW_Q"], True)
        m["w_vK"] = pair_T(xs["vW_K"], True)
        m["w_eQ"] = pair_T(xs["eW_Q"], True)
        m["w_eK"] = pair_T(xs["eW_K"], True)
        m["w_eV"] = pair_T(xs["eW_V"], False)
        m["w_fK"] = pair_T(xs["fW_K"], True)
        m["w_lv"] = pair_T(xs["vW_V"], False)
        m["w_ov"] = out_w(xs["vW_out"])
        m["w_oe"] = out_w(xs["eW_out"])
        for nm in ("vQ", "vK", "eQ", "eK", "fK"):
            m["g_" + nm], m["b_" + nm] = lnvec(nm)
        m["ones_sq"] = ones_sq
        m["bc2"] = bc2
        m["bc128"] = bc128
        in_maps.append(m)
    return in_maps, gidx, xs


def kernel(**inputs):
    nc = _get_nc()
    in_maps, gidx, xs = _prep_host(inputs)
    res = run_bass_kernel_spmd(nc, in_maps, core_ids=list(range(8))).results

    out_v = np.zeros((BS, N, DV), np.float32)
    out_e = np.zeros((BS, M, DV), np.float32)
    for b in range(BS):
        pv = np.zeros((N, DV), np.float32)
        for c in range(4 * b, 4 * b + 4):
            out_v[b] += res[c]["outv"]
            out_e[b] += res[c]["oute"]
            pv += res[c]["pv"]
        ip, im = gidx[b]
        out_e[b] += pv[ip] - pv[im]
        out_v[b] += np.asarray(xs["vb_out"])[None, :]
        out_e[b] += np.asarray(xs["eb_out"])[None, :]
    return out_v, out_e
